# revision 18
# baseline (speedup 1.0000x reference)
"""BailingMoE linear attention block on 8 trn2 cores (tensor-parallel over heads).

v3: bf16 datapath, single-pass projections, and slice/chunk interleaving.

Per rank r of 8 (heads 2r, 2r+1; 256 of 2048 feature columns):
  - one pass over hidden^T (bf16): feature-major q,k,g projections and
    natural-layout v projection from the same SBUF-resident h tiles.
  - RoPE: k on DVE straight out of PSUM, q on Pool from an Act-drained
    scratch copy (Pool cannot read PSUM).
  - chunked linear attention (B=512) interleaved with the projections:
    slice c is projected, then chunk c's attention runs while slice c+1's
    matmuls keep the PE busy, so the PSUM-drain engines (DVE/Act) are never
    the critical path. The 8 PSUM banks are shared between the two roles
    via common pool tags.
  - triangular-trimmed scores/outputs, PE transposes of k for the state
    update, bf16 small matmuls, per-head decay state S [128,128] bf16.
  - fused dense (row slice of w_dense, g_norm pre-folded), y partial bf16;
    dense for chunk c-1 is emitted inside chunk c (software pipeline).
  - partial sum-of-squares of o emitted as ssq [1, T] f32.
Host: y = (sum_r y_r) * rsqrt(sum_r ssq_r / 2048 + eps) -- the RMSNorm scale
commutes through the dense matmul, so no on-device collective is needed.
"""
import numpy as np
import ml_dtypes
import concourse.bass as bass
import concourse.mybir as mybir
import concourse.tile as tile
from concourse import bacc, bass_utils

T, HID, H, D = 4096, 2048, 16, 128
EPS = 1e-5
THETA = 600000.0
NUM_LAYERS, LAYER_ID = 32, 0
M = 8                 # cores
HPR = H // M          # heads per rank = 2
CW = HPR * D          # feature columns per rank = 256
NK = HID // 128       # 16 k-tiles
TT = 512              # t-slice for projections
NT = T // TT          # 8
B2 = 512              # attention chunk
NCH = T // B2         # 8

f32 = mybir.dt.float32
bf16 = mybir.dt.bfloat16
NPBF16 = ml_dtypes.bfloat16

_PROGRAM = None


def _build_program():
    nc = bacc.Bacc(trn_type="TRN2")

    hTb = nc.dram_tensor("hTb", [128, NK, T], bf16, kind="ExternalInput")
    w_all = nc.dram_tensor("w_all", [128, NK, 1024], bf16, kind="ExternalInput")
    wdd = nc.dram_tensor("wdd", [128, HPR, HID], bf16, kind="ExternalInput")
    cosT = nc.dram_tensor("cosT", [D, T], bf16, kind="ExternalInput")
    sinT = nc.dram_tensor("sinT", [D, T], bf16, kind="ExternalInput")
    maskT = nc.dram_tensor("maskT", [128, HPR * 4 * B2], bf16, kind="ExternalInput")
    qdtab = nc.dram_tensor("qdtab", [128, HPR * B2], bf16, kind="ExternalInput")
    kdcol = nc.dram_tensor("kdcol", [128, HPR * 4], f32, kind="ExternalInput")
    bdcol = nc.dram_tensor("bdcol", [128, HPR], f32, kind="ExternalInput")

    y_nat = nc.dram_tensor("y_nat", [T, HID], bf16, kind="ExternalOutput")
    ssq = nc.dram_tensor("ssq", [1, T], f32, kind="ExternalOutput")

    ACT = mybir.ActivationFunctionType
    ALU = mybir.AluOpType
    half = D // 2

    with tile.TileContext(nc) as tc:
        with tc.tile_pool(name="persist", bufs=1) as persist:
            qTb = persist.tile([128, HPR, T], bf16, name="qTb")
            kTb = persist.tile([128, HPR, T], bf16, name="kTb")
            vN = persist.tile([128, NCH * 4, CW], bf16, name="vN")
            gTb = persist.tile([128, HPR, T], bf16, name="gTb")
            S = [persist.tile([128, 128], bf16, name=f"S{h}") for h in range(HPR)]
            mk_sb = persist.tile([128, HPR * 4 * B2], bf16, name="mk_sb")
            qd_sb = persist.tile([128, HPR * B2], bf16, name="qd_sb")
            kd_sb = persist.tile([128, HPR * 4], f32, name="kd_sb")
            bd_sb = persist.tile([128, HPR], f32, name="bd_sb")
            wd_sb = persist.tile([128, HPR, HID], bf16, name="wd_sb")
            ident_f = persist.tile([128, 128], f32, name="ident_f")
            identb = persist.tile([128, 128], bf16, name="identb")
            onesb = persist.tile([128, 1], bf16, name="onesb")

            from concourse.masks import make_identity

            make_identity(nc, ident_f[:])
            nc.vector.tensor_copy(identb[:], ident_f[:])
            nc.gpsimd.memset(onesb[:], 1.0)
            for h in range(HPR):
                nc.vector.memset(S[h][:], 0.0)

            with (
                tc.tile_pool(name="work", bufs=1) as work,
                tc.tile_pool(name="psAB", bufs=1, space="PSUM") as psAB,
            ):
                wpool = astream = ascr = bscr2 = work
                w_sb = wpool.tile([128, NK, 1024], bf16, name="w_sb")

                # ---------------- phase A slice ------------------------------
                def emit_slice(t):
                    tsl = bass.ds(t * TT, TT)
                    hk = astream.tile([128, NK, TT], bf16, tag="hk", bufs=2, name=f"hk{t}")
                    if t == 0:
                        # first slice: SP streams k0-7, Act streams k8-15 in
                        # parallel so the fg chains are fed with minimal lag
                        for half_eng, base in ((nc.sync, 0), (nc.scalar, 8)):
                            for k2 in range(base, base + 8, 2):
                                half_eng.dma_start(
                                    hk[:, k2:k2 + 2, :],
                                    hTb[:, k2:k2 + 2, t * TT:(t + 1) * TT],
                                )
                                if k2 % 4 == 0:
                                    half_eng.dma_start(
                                        w_sb[:, k2:k2 + 4, :],
                                        w_all[:, k2:k2 + 4, :],
                                    )
                    else:
                        for k4 in range(0, NK, 4):
                            nc.sync.dma_start(
                                hk[:, k4:k4 + 4, :],
                                hTb[:, k4:k4 + 4, t * TT:(t + 1) * TT],
                            )
                    cs = astream.tile([128, TT], bf16, tag="cs", bufs=2, name=f"cs{t}")
                    nc.scalar.dma_start(cs[:], cosT[:, t * TT:(t + 1) * TT])
                    sn = astream.tile([128, TT], bf16, tag="sn", bufs=2, name=f"sn{t}")
                    nc.scalar.dma_start(sn[:], sinT[:, t * TT:(t + 1) * TT])

                    # feature-major: q0 q1 k0 k1 g0 g1 on bank tags B0..B5
                    for fg in range(6):
                        acc = psAB.tile(
                            [128, TT], f32, tag=f"B{fg}", name=f"A{t}_{fg}"
                        )
                        for k in range(NK):
                            nc.tensor.matmul(
                                acc[:],
                                w_sb[:, k, fg * 128:(fg + 1) * 128],
                                hk[:, k, :],
                                start=(k == 0),
                                stop=(k == NK - 1),
                            )
                        if fg < 4:
                            # rope via Act-drained scratch: q on DVE, k on Pool
                            h = fg % 2
                            scr = ascr.tile(
                                [128, TT], bf16, tag=f"scr{fg}", bufs=2,
                                name=f"scr{t}_{fg}"
                            )
                            nc.scalar.copy(scr[:], acc[:])
                            eng = nc.vector if fg < 2 else nc.gpsimd
                            en = "v" if fg < 2 else "p"
                            dst = qTb if fg < 2 else kTb
                            t1 = ascr.tile(
                                [128, TT], bf16, tag=f"t1{en}", name=f"t1{t}_{fg}"
                            )
                            tmp = ascr.tile(
                                [128, TT], bf16, tag=f"tm{en}", name=f"tm{t}_{fg}"
                            )
                            eng.tensor_mul(t1[:], scr[:], cs[:])
                            eng.tensor_mul(
                                tmp[0:half, :], scr[half:D, :], sn[half:D, :]
                            )
                            eng.tensor_mul(
                                tmp[half:D, :], scr[0:half, :], sn[0:half, :]
                            )
                            eng.tensor_add(dst[:, h, tsl], t1[:], tmp[:])
                        else:
                            nc.scalar.activation(
                                gTb[:, fg - 4, tsl], acc[:], ACT.Sigmoid
                            )

                    # natural-layout v (both heads' 256 features at once)
                    vb0 = psAB.tile([128, 512], f32, tag="B6", name=f"VB0_{t}")
                    vb1 = psAB.tile([128, 512], f32, tag="B7", name=f"VB1_{t}")
                    accv = [
                        vb0[:, 0:256], vb0[:, 256:512],
                        vb1[:, 0:256], vb1[:, 256:512],
                    ]
                    for tb in range(4):
                        for k in range(NK):
                            nc.tensor.matmul(
                                accv[tb],
                                hk[:, k, tb * 128:(tb + 1) * 128],
                                w_sb[:, k, 768:1024],
                                start=(k == 0),
                                stop=(k == NK - 1),
                            )
                        nc.scalar.copy(vN[:, t * 4 + tb, :], accv[tb])

                # ---------------- phase B chunk ------------------------------
                def scores_stage(c, h):
                    """PE: scores (triangular) + k transposes."""
                    nm = f"{c}_{h}"
                    qs = qTb[:, h, c * B2:(c + 1) * B2]
                    pta = psAB.tile([128, 512], f32, tag="B0", name=f"pta{nm}")
                    ptb = psAB.tile([128, 512], f32, tag="B1", name=f"ptb{nm}")
                    ptc = psAB.tile([128, 512], f32, tag="B2", name=f"ptc{nm}")
                    tkd = psAB.tile([128, 512], bf16, tag="B3", name=f"tkd{nm}")
                    pts = [
                        pta[:, 0:512], ptb[:, 0:384],
                        ptc[:, 0:256], ptb[:, 384:512],
                    ]
                    tkl = [tkd[:, jh * 128:(jh + 1) * 128] for jh in range(4)]
                    for jh in range(4):
                        nc.tensor.matmul(
                            pts[jh],
                            kTb[:, h, c * B2 + jh * 128: c * B2 + (jh + 1) * 128],
                            qs[:, jh * 128:B2],
                            start=True,
                            stop=True,
                        )
                    for jh in range(4):
                        nc.tensor.transpose(
                            tkl[jh],
                            kTb[:, h, c * B2 + jh * 128: c * B2 + (jh + 1) * 128],
                            identb[:],
                        )
                    return pts, tkl

                def mask_stage(c, h, pts):
                    """DVE: masked scores -> bf16 (trimmed)."""
                    nm = f"{c}_{h}"
                    m1 = []
                    for jh in range(4):
                        w = B2 - jh * 128
                        m1t = bscr2.tile(
                            [128, w], bf16, tag=f"M1_{jh}", bufs=2,
                            name=f"m1{nm}_{jh}"
                        )
                        nc.vector.tensor_mul(
                            m1t[:, 0:w],
                            pts[jh],
                            mk_sb[:, (h * 4 + jh) * B2 + jh * 128:
                                  (h * 4 + jh + 1) * B2],
                        )
                        m1.append(m1t)
                    return m1

                def kns_stage(c, h, tkl):
                    """Act: scaled natural-layout k -> bf16."""
                    nm = f"{c}_{h}"
                    kns = []
                    for jh in range(4):
                        knt = bscr2.tile(
                            [128, 128], bf16, tag=f"KN_{jh}", bufs=2,
                            name=f"kn{nm}_{jh}"
                        )
                        nc.scalar.activation(
                            knt[:], tkl[jh], ACT.Copy,
                            scale=kd_sb[:, h * 4 + jh: h * 4 + jh + 1],
                        )
                        kns.append(knt)
                    return kns

                def qp_stage(c, h):
                    nm = f"{c}_{h}"
                    qp = bscr2.tile([128, B2], bf16, tag=f"QP{h}", bufs=2, name=f"qp{nm}")
                    nc.gpsimd.tensor_mul(
                        qp[:], qTb[:, h, c * B2:(c + 1) * B2],
                        qd_sb[:, h * B2:(h + 1) * B2],
                    )
                    return qp

                def dense_half(cp, xc, tbs):
                    """Dense for chunk cp over token subtiles tbs."""
                    for tb in tbs:
                        dsb = bscr2.tile(
                            [128, HID], bf16, tag=f"DSB{tb}", bufs=1,
                            name=f"dsb{cp}_{tb}",
                        )
                        for ms in range(4):
                            dps = psAB.tile(
                                [128, 512], f32, tag=f"B{6 + ms % 2}",
                                name=f"d{cp}_{tb}_{ms}",
                            )
                            for kf in range(HPR):
                                nc.tensor.matmul(
                                    dps[:],
                                    xc[kf][:, tb * 128:(tb + 1) * 128],
                                    wd_sb[:, kf, ms * 512:(ms + 1) * 512],
                                    start=(kf == 0),
                                    stop=(kf == HPR - 1),
                                )
                            idx = tb * 4 + ms
                            if idx in (0, 3, 6, 9, 12, 15):
                                nc.vector.tensor_copy(
                                    dsb[:, ms * 512:(ms + 1) * 512], dps[:]
                                )
                            else:
                                nc.scalar.copy(
                                    dsb[:, ms * 512:(ms + 1) * 512], dps[:]
                                )
                            if cp == NCH - 1:
                                nc.sync.dma_start(
                                    y_nat[(cp * 4 + tb) * 128:
                                          (cp * 4 + tb + 1) * 128,
                                          ms * 512:(ms + 1) * 512],
                                    dsb[:, ms * 512:(ms + 1) * 512],
                                )
                        if cp != NCH - 1:
                            nc.sync.dma_start(
                                y_nat[(cp * 4 + tb) * 128:(cp * 4 + tb + 1) * 128, :],
                                dsb[:],
                            )

                def out_stage(c, h, m1, kns, qp, obq):
                    """PE o + sps; Act sq; DVE xch + S update."""
                    nm = f"{c}_{h}"
                    ob = psAB.tile([128, 512], f32, tag="B4", name=f"ob{nm}")
                    for ih in range(4):
                        osl = ob[:, ih * 128:(ih + 1) * 128]
                        nc.tensor.matmul(
                            osl, S[h][:], qp[:, ih * 128:(ih + 1) * 128],
                            start=True, stop=False,
                        )
                        for jh in range(ih + 1):
                            nc.tensor.matmul(
                                osl,
                                vN[:, c * 4 + jh, h * 128:(h + 1) * 128],
                                m1[jh][:, (ih - jh) * 128:(ih - jh + 1) * 128],
                                start=False,
                                stop=(jh == ih),
                            )
                    spst = psAB.tile([128, 512], f32, tag="B5", name=f"sps{nm}")
                    sps_ps = spst[:, 0:128]
                    for jh in range(4):
                        nc.tensor.matmul(
                            sps_ps, kns[jh][:],
                            vN[:, c * 4 + jh, h * 128:(h + 1) * 128],
                            start=(jh == 0), stop=(jh == 3),
                        )
                    sq = bscr2.tile([128, B2], bf16, tag="SQ", bufs=2, name=f"sq{nm}")
                    nc.scalar.square(sq[:], ob[:])
                    xch = bscr2.tile([128, B2], bf16, tag=f"XC{h}", bufs=2, name=f"xc{nm}")
                    nc.vector.tensor_mul(xch[:], ob[:], gTb[:, h, c * B2:(c + 1) * B2])
                    nc.vector.scalar_tensor_tensor(
                        out=S[h][:],
                        in0=S[h][:],
                        scalar=bd_sb[:, h: h + 1],
                        in1=sps_ps,
                        op0=ALU.mult,
                        op1=ALU.add,
                    )
                    obq.append(sq)
                    return xch

                for t in range(NT):
                    emit_slice(t)
                    if t == 0:
                        # table loads ride the Act queue once the h/w stream
                        # for the first slice has been issued on SP
                        nc.scalar.dma_start(mk_sb[:], maskT[:, :])
                        nc.scalar.dma_start(qd_sb[:], qdtab[:, :])
                        nc.scalar.dma_start(kd_sb[:], kdcol[:, :])
                        nc.scalar.dma_start(bd_sb[:], bdcol[:, :])
                        nc.scalar.dma_start(wd_sb[:], wdd[:, :, :])

                prev_xc = None
                for c in range(NCH):
                    obq = []
                    pts0, tkl0 = scores_stage(c, 0)
                    m10 = mask_stage(c, 0, pts0)
                    kns0 = kns_stage(c, 0, tkl0)
                    qp0 = qp_stage(c, 0)
                    if prev_xc is not None:
                        dense_half(prev_xc[0], prev_xc[1], [0, 1])
                    pts1, tkl1 = scores_stage(c, 1)
                    m11 = mask_stage(c, 1, pts1)
                    kns1 = kns_stage(c, 1, tkl1)
                    qp1 = qp_stage(c, 1)
                    if prev_xc is not None:
                        dense_half(prev_xc[0], prev_xc[1], [2, 3])
                    xc0 = out_stage(c, 0, m10, kns0, qp0, obq)
                    xc1 = out_stage(c, 1, m11, kns1, qp1, obq)
                    # ssq: accumulate both heads via ones-matmul (B5 bank reuse)
                    sqps = psAB.tile([1, 512], f32, tag="B5", name=f"sqps{c}")
                    for h in range(HPR):
                        nc.tensor.matmul(
                            sqps[:], onesb[:], obq[h][:],
                            start=(h == 0), stop=(h == HPR - 1),
                        )
                    ssqt = bscr2.tile([1, 512], f32, tag="SSQ", bufs=1, name=f"ssqt{c}")
                    nc.scalar.copy(ssqt[:], sqps[:])
                    nc.sync.dma_start(ssq[:, c * B2:(c + 1) * B2], ssqt[:])
                    prev_xc = (c, [xc0, xc1])

                dense_half(prev_xc[0], prev_xc[1], [0, 1])
                dense_half(prev_xc[0], prev_xc[1], [2, 3])

    nc.compile()
    return nc


def _slopes(n):
    start = 2.0 ** (-(2.0 ** -(np.log2(n) - 3)))
    return np.array([start ** (i + 1) for i in range(n)], dtype=np.float64)


def kernel(hidden_states, positions, w_qkv, w_g, w_dense, g_norm_weight):
    global _PROGRAM
    if _PROGRAM is None:
        _PROGRAM = _build_program()
    nc = _PROGRAM

    hidden_states = np.asarray(hidden_states, dtype=np.float32)
    positions = np.asarray(positions)
    w_qkv = np.asarray(w_qkv, dtype=np.float32)
    w_g = np.asarray(w_g, dtype=np.float32)
    w_dense = np.asarray(w_dense, dtype=np.float32)
    g_norm_weight = np.asarray(g_norm_weight, dtype=np.float32)

    hT = hidden_states.T.reshape(NK, 128, T).transpose(1, 0, 2)
    hTb = np.ascontiguousarray(hT).astype(NPBF16)

    # rope tables, feature-major; sinT carries the rotate-half signs
    half = D // 2
    inv_freq = 1.0 / (THETA ** (np.arange(0, D, 2, dtype=np.float64) / D))
    freqs = positions.astype(np.float64)[:, None] * inv_freq          # [T, 64]
    cos = np.cos(freqs).T                                             # [64, T]
    sin = np.sin(freqs).T
    cosT = np.concatenate([cos, cos], axis=0).astype(NPBF16)          # [128, T]
    sinT = np.concatenate([sin, -sin], axis=0).astype(NPBF16)

    s = _slopes(H) * (1.0 - LAYER_ID / (NUM_LAYERS - 1) + 1e-5)       # [16]
    idx = np.arange(B2, dtype=np.float64)
    diff = idx[:, None] - idx[None, :]
    scale = D ** -0.5
    decay = np.where(
        diff[None, :, :] >= 0, np.exp(-s[:, None, None] * diff[None, :, :]), 0.0
    )                                                                  # [16, B2, B2]
    qd = np.exp(-s[:, None] * (idx[None, :] + 1.0)) * scale            # [16, B2]
    kd = np.exp(-s[:, None] * (B2 - 1.0 - idx[None, :]))               # [16, B2]
    bd = np.exp(-s * B2)                                               # [16]

    in_maps = []
    for r in range(M):
        heads = [HPR * r + i for i in range(HPR)]
        cols = slice(r * CW, (r + 1) * CW)
        wq = w_qkv[:, r * CW:(r + 1) * CW]
        wk = w_qkv[:, HID + r * CW: HID + (r + 1) * CW]
        wv = w_qkv[:, 2 * HID + r * CW: 2 * HID + (r + 1) * CW]
        wg = w_g[:, cols]
        wcat = np.concatenate([wq, wk, wg, wv], axis=1)               # [HID, 1024]
        w_allr = np.ascontiguousarray(
            wcat.reshape(NK, 128, 1024).transpose(1, 0, 2)
        ).astype(NPBF16)
        wdr = (g_norm_weight[cols, None] * w_dense[cols, :])
        wdr = np.ascontiguousarray(
            wdr.reshape(HPR, 128, HID).transpose(1, 0, 2)
        ).astype(NPBF16)

        mk = np.empty((128, HPR * 4 * B2), np.float64)
        qdt = np.empty((128, HPR * B2), np.float64)
        kdc = np.empty((128, HPR * 4), np.float32)
        bdc = np.empty((128, HPR), np.float32)
        for i, h in enumerate(heads):
            mTh = decay[h].T * scale                                   # [j, i]
            for jh in range(4):
                mk[:, (i * 4 + jh) * B2:(i * 4 + jh + 1) * B2] = (
                    mTh[jh * 128:(jh + 1) * 128, :]
                )
                kdc[:, i * 4 + jh] = kd[h, jh * 128:(jh + 1) * 128]
            qdt[:, i * B2:(i + 1) * B2] = np.broadcast_to(
                qd[h][None, :], (128, B2)
            )
            bdc[:, i] = bd[h]

        in_maps.append(
            {
                "hTb": hTb,
                "w_all": w_allr,
                "wdd": wdr,
                "cosT": cosT,
                "sinT": sinT,
                "maskT": mk.astype(NPBF16),
                "qdtab": qdt.astype(NPBF16),
                "kdcol": kdc,
                "bdcol": bdc,
            }
        )

    global _LAST_IN_MAPS
    _LAST_IN_MAPS = in_maps
    results = bass_utils.run_bass_kernel_spmd(nc, in_maps, core_ids=list(range(M)))

    y_sum = np.zeros((T, HID), np.float64)
    ssq_tot = np.zeros((T,), np.float64)
    for r in range(M):
        y_sum += results.results[r]["y_nat"].astype(np.float64)
        ssq_tot += results.results[r]["ssq"][0].astype(np.float64)
    var = ssq_tot / (H * D)
    F = 1.0 / np.sqrt(var + EPS)
    y = y_sum * F[:, None]
    return y.astype(np.float32)


# revision 20
# speedup vs baseline: 1.0141x; 1.0141x over previous
"""BailingMoE linear attention block on 8 trn2 cores (tensor-parallel over heads).

v3: bf16 datapath, single-pass projections, and slice/chunk interleaving.

Per rank r of 8 (heads 2r, 2r+1; 256 of 2048 feature columns):
  - one pass over hidden^T (bf16): feature-major q,k,g projections and
    natural-layout v projection from the same SBUF-resident h tiles.
  - RoPE: k on DVE straight out of PSUM, q on Pool from an Act-drained
    scratch copy (Pool cannot read PSUM).
  - chunked linear attention (B=512) interleaved with the projections:
    slice c is projected, then chunk c's attention runs while slice c+1's
    matmuls keep the PE busy, so the PSUM-drain engines (DVE/Act) are never
    the critical path. The 8 PSUM banks are shared between the two roles
    via common pool tags.
  - triangular-trimmed scores/outputs, PE transposes of k for the state
    update, bf16 small matmuls, per-head decay state S [128,128] bf16.
  - fused dense (row slice of w_dense, g_norm pre-folded), y partial bf16;
    dense for chunk c-1 is emitted inside chunk c (software pipeline).
  - partial sum-of-squares of o emitted as ssq [1, T] f32.
Host: y = (sum_r y_r) * rsqrt(sum_r ssq_r / 2048 + eps) -- the RMSNorm scale
commutes through the dense matmul, so no on-device collective is needed.
"""
import numpy as np
import ml_dtypes
import concourse.bass as bass
import concourse.mybir as mybir
import concourse.tile as tile
from concourse import bacc, bass_utils

T, HID, H, D = 4096, 2048, 16, 128
EPS = 1e-5
THETA = 600000.0
NUM_LAYERS, LAYER_ID = 32, 0
M = 8                 # cores
HPR = H // M          # heads per rank = 2
CW = HPR * D          # feature columns per rank = 256
NK = HID // 128       # 16 k-tiles
TT = 512              # t-slice for projections
NT = T // TT          # 8
B2 = 512              # attention chunk
NCH = T // B2         # 8

f32 = mybir.dt.float32
bf16 = mybir.dt.bfloat16
NPBF16 = ml_dtypes.bfloat16

_PROGRAM = None


def _build_program():
    nc = bacc.Bacc(trn_type="TRN2")

    hTb = nc.dram_tensor("hTb", [128, NK, T], bf16, kind="ExternalInput")
    w_all = nc.dram_tensor("w_all", [128, NK, 1024], bf16, kind="ExternalInput")
    wdd = nc.dram_tensor("wdd", [128, HPR, HID], bf16, kind="ExternalInput")
    cosT = nc.dram_tensor("cosT", [D, T], bf16, kind="ExternalInput")
    sinT = nc.dram_tensor("sinT", [D, T], bf16, kind="ExternalInput")
    maskT = nc.dram_tensor("maskT", [128, HPR * 4 * B2], bf16, kind="ExternalInput")
    qdtab = nc.dram_tensor("qdtab", [128, HPR * B2], bf16, kind="ExternalInput")
    kdcol = nc.dram_tensor("kdcol", [128, HPR * 4], f32, kind="ExternalInput")
    bdcol = nc.dram_tensor("bdcol", [128, HPR], f32, kind="ExternalInput")

    y_nat = nc.dram_tensor("y_nat", [T, HID], bf16, kind="ExternalOutput")
    ssq = nc.dram_tensor("ssq", [1, T], f32, kind="ExternalOutput")

    ACT = mybir.ActivationFunctionType
    ALU = mybir.AluOpType
    half = D // 2

    with tile.TileContext(nc) as tc:
        with tc.tile_pool(name="persist", bufs=1) as persist:
            qTb = persist.tile([128, HPR, T], bf16, name="qTb")
            kTb = persist.tile([128, HPR, T], bf16, name="kTb")
            vN = persist.tile([128, NCH * 4, CW], bf16, name="vN")
            gTb = persist.tile([128, HPR, T], bf16, name="gTb")
            S = [persist.tile([128, 128], bf16, name=f"S{h}") for h in range(HPR)]
            mk_sb = persist.tile([128, HPR * 4 * B2], bf16, name="mk_sb")
            qd_sb = persist.tile([128, HPR * B2], bf16, name="qd_sb")
            kd_sb = persist.tile([128, HPR * 4], f32, name="kd_sb")
            bd_sb = persist.tile([128, HPR], f32, name="bd_sb")
            wd_sb = persist.tile([128, HPR, HID], bf16, name="wd_sb")
            ident_f = persist.tile([128, 128], f32, name="ident_f")
            identb = persist.tile([128, 128], bf16, name="identb")
            onesb = persist.tile([128, 1], bf16, name="onesb")

            from concourse.masks import make_identity

            make_identity(nc, ident_f[:])
            nc.vector.tensor_copy(identb[:], ident_f[:])
            nc.gpsimd.memset(onesb[:], 1.0)
            for h in range(HPR):
                nc.vector.memset(S[h][:], 0.0)

            with (
                tc.tile_pool(name="work", bufs=1) as work,
                tc.tile_pool(name="psAB", bufs=1, space="PSUM") as psAB,
            ):
                wpool = astream = ascr = bscr2 = work
                w_sb = wpool.tile([128, NK, 1024], bf16, name="w_sb")

                # ---------------- phase A slice ------------------------------
                def emit_slice(t):
                    tsl = bass.ds(t * TT, TT)
                    hk = astream.tile([128, NK, TT], bf16, tag="hk", bufs=2, name=f"hk{t}")
                    if t == 0:
                        # first slice: per-k (h, w) pairs so the k-major
                        # matmul order below starts after ~0.4 MB of traffic
                        for k in range(NK):
                            nc.sync.dma_start(
                                hk[:, k:k + 1, :],
                                hTb[:, k:k + 1, t * TT:(t + 1) * TT],
                            )
                            nc.sync.dma_start(
                                w_sb[:, k:k + 1, :], w_all[:, k:k + 1, :]
                            )
                    else:
                        for k4 in range(0, NK, 4):
                            nc.sync.dma_start(
                                hk[:, k4:k4 + 4, :],
                                hTb[:, k4:k4 + 4, t * TT:(t + 1) * TT],
                            )
                    cs = astream.tile([128, TT], bf16, tag="cs", bufs=2, name=f"cs{t}")
                    nc.scalar.dma_start(cs[:], cosT[:, t * TT:(t + 1) * TT])
                    sn = astream.tile([128, TT], bf16, tag="sn", bufs=2, name=f"sn{t}")
                    nc.scalar.dma_start(sn[:], sinT[:, t * TT:(t + 1) * TT])

                    # feature-major: q0 q1 k0 k1 g0 g1 on bank tags B0..B5
                    accs = [
                        psAB.tile([128, TT], f32, tag=f"B{fg}", name=f"A{t}_{fg}")
                        for fg in range(6)
                    ]
                    vb0 = psAB.tile([128, 512], f32, tag="B6", name=f"VB0_{t}")
                    vb1 = psAB.tile([128, 512], f32, tag="B7", name=f"VB1_{t}")
                    accv = [
                        vb0[:, 0:256], vb0[:, 256:512],
                        vb1[:, 0:256], vb1[:, 256:512],
                    ]
                    if t == 0:
                        # k-major: each k-step only needs hk[k] + w[k].
                        # (v matmuls stay fg-major below: accv pairs share a
                        # PSUM bank, and accumulation groups in one bank
                        # cannot interleave.)
                        for k in range(NK):
                            for fg in range(6):
                                nc.tensor.matmul(
                                    accs[fg][:],
                                    w_sb[:, k, fg * 128:(fg + 1) * 128],
                                    hk[:, k, :],
                                    start=(k == 0),
                                    stop=(k == NK - 1),
                                )
                    for fg in range(6):
                        acc = accs[fg]
                        if t > 0:
                            for k in range(NK):
                                nc.tensor.matmul(
                                    acc[:],
                                    w_sb[:, k, fg * 128:(fg + 1) * 128],
                                    hk[:, k, :],
                                    start=(k == 0),
                                    stop=(k == NK - 1),
                                )
                        if fg < 4:
                            # rope via Act-drained scratch: q on DVE, k on Pool
                            h = fg % 2
                            scr = ascr.tile(
                                [128, TT], bf16, tag=f"scr{fg}", bufs=2,
                                name=f"scr{t}_{fg}"
                            )
                            nc.scalar.copy(scr[:], acc[:])
                            eng = nc.vector if fg < 2 else nc.gpsimd
                            en = "v" if fg < 2 else "p"
                            dst = qTb if fg < 2 else kTb
                            t1 = ascr.tile(
                                [128, TT], bf16, tag=f"t1{en}", name=f"t1{t}_{fg}"
                            )
                            tmp = ascr.tile(
                                [128, TT], bf16, tag=f"tm{en}", name=f"tm{t}_{fg}"
                            )
                            eng.tensor_mul(t1[:], scr[:], cs[:])
                            eng.tensor_mul(
                                tmp[0:half, :], scr[half:D, :], sn[half:D, :]
                            )
                            eng.tensor_mul(
                                tmp[half:D, :], scr[0:half, :], sn[0:half, :]
                            )
                            eng.tensor_add(dst[:, h, tsl], t1[:], tmp[:])
                        else:
                            nc.scalar.activation(
                                gTb[:, fg - 4, tsl], acc[:], ACT.Sigmoid
                            )

                    # natural-layout v (both heads' 256 features at once)
                    for tb in range(4):
                        for k in range(NK):
                            nc.tensor.matmul(
                                accv[tb],
                                hk[:, k, tb * 128:(tb + 1) * 128],
                                w_sb[:, k, 768:1024],
                                start=(k == 0),
                                stop=(k == NK - 1),
                            )
                        nc.scalar.copy(vN[:, t * 4 + tb, :], accv[tb])

                # ---------------- phase B chunk ------------------------------
                def scores_stage(c, h):
                    """PE: scores (triangular) + k transposes."""
                    nm = f"{c}_{h}"
                    qs = qTb[:, h, c * B2:(c + 1) * B2]
                    pta = psAB.tile([128, 512], f32, tag="B0", name=f"pta{nm}")
                    ptb = psAB.tile([128, 512], f32, tag="B1", name=f"ptb{nm}")
                    ptc = psAB.tile([128, 512], f32, tag="B2", name=f"ptc{nm}")
                    tkd = psAB.tile([128, 512], bf16, tag="B3", name=f"tkd{nm}")
                    pts = [
                        pta[:, 0:512], ptb[:, 0:384],
                        ptc[:, 0:256], ptb[:, 384:512],
                    ]
                    tkl = [tkd[:, jh * 128:(jh + 1) * 128] for jh in range(4)]
                    for jh in range(4):
                        nc.tensor.matmul(
                            pts[jh],
                            kTb[:, h, c * B2 + jh * 128: c * B2 + (jh + 1) * 128],
                            qs[:, jh * 128:B2],
                            start=True,
                            stop=True,
                        )
                    for jh in range(4):
                        nc.tensor.transpose(
                            tkl[jh],
                            kTb[:, h, c * B2 + jh * 128: c * B2 + (jh + 1) * 128],
                            identb[:],
                        )
                    return pts, tkl

                def mask_stage(c, h, pts):
                    """DVE: masked scores -> bf16 (trimmed)."""
                    nm = f"{c}_{h}"
                    m1 = []
                    for jh in range(4):
                        w = B2 - jh * 128
                        m1t = bscr2.tile(
                            [128, w], bf16, tag=f"M1_{jh}", bufs=2,
                            name=f"m1{nm}_{jh}"
                        )
                        nc.vector.tensor_mul(
                            m1t[:, 0:w],
                            pts[jh],
                            mk_sb[:, (h * 4 + jh) * B2 + jh * 128:
                                  (h * 4 + jh + 1) * B2],
                        )
                        m1.append(m1t)
                    return m1

                def kns_stage(c, h, tkl):
                    """Act: scaled natural-layout k -> bf16."""
                    nm = f"{c}_{h}"
                    kns = []
                    for jh in range(4):
                        knt = bscr2.tile(
                            [128, 128], bf16, tag=f"KN_{jh}", bufs=2,
                            name=f"kn{nm}_{jh}"
                        )
                        nc.scalar.activation(
                            knt[:], tkl[jh], ACT.Copy,
                            scale=kd_sb[:, h * 4 + jh: h * 4 + jh + 1],
                        )
                        kns.append(knt)
                    return kns

                def qp_stage(c, h):
                    nm = f"{c}_{h}"
                    qp = bscr2.tile([128, B2], bf16, tag=f"QP{h}", bufs=2, name=f"qp{nm}")
                    nc.gpsimd.tensor_mul(
                        qp[:], qTb[:, h, c * B2:(c + 1) * B2],
                        qd_sb[:, h * B2:(h + 1) * B2],
                    )
                    return qp

                def dense_half(cp, xc, tbs):
                    """Dense for chunk cp over token subtiles tbs."""
                    for tb in tbs:
                        dsb = bscr2.tile(
                            [128, HID], bf16, tag=f"DSB{tb}", bufs=1,
                            name=f"dsb{cp}_{tb}",
                        )
                        for ms in range(4):
                            dps = psAB.tile(
                                [128, 512], f32, tag=f"B{6 + ms % 2}",
                                name=f"d{cp}_{tb}_{ms}",
                            )
                            for kf in range(HPR):
                                nc.tensor.matmul(
                                    dps[:],
                                    xc[kf][:, tb * 128:(tb + 1) * 128],
                                    wd_sb[:, kf, ms * 512:(ms + 1) * 512],
                                    start=(kf == 0),
                                    stop=(kf == HPR - 1),
                                )
                            idx = tb * 4 + ms
                            if idx in (0, 3, 6, 9, 12, 15):
                                nc.vector.tensor_copy(
                                    dsb[:, ms * 512:(ms + 1) * 512], dps[:]
                                )
                            else:
                                nc.scalar.copy(
                                    dsb[:, ms * 512:(ms + 1) * 512], dps[:]
                                )
                            if cp == NCH - 1:
                                nc.sync.dma_start(
                                    y_nat[(cp * 4 + tb) * 128:
                                          (cp * 4 + tb + 1) * 128,
                                          ms * 512:(ms + 1) * 512],
                                    dsb[:, ms * 512:(ms + 1) * 512],
                                )
                        if cp != NCH - 1:
                            nc.sync.dma_start(
                                y_nat[(cp * 4 + tb) * 128:(cp * 4 + tb + 1) * 128, :],
                                dsb[:],
                            )

                def out_stage(c, h, m1, kns, qp, obq):
                    """PE o + sps; Act sq; DVE xch + S update."""
                    nm = f"{c}_{h}"
                    ob = psAB.tile([128, 512], f32, tag="B4", name=f"ob{nm}")
                    for ih in range(4):
                        osl = ob[:, ih * 128:(ih + 1) * 128]
                        nc.tensor.matmul(
                            osl, S[h][:], qp[:, ih * 128:(ih + 1) * 128],
                            start=True, stop=False,
                        )
                        for jh in range(ih + 1):
                            nc.tensor.matmul(
                                osl,
                                vN[:, c * 4 + jh, h * 128:(h + 1) * 128],
                                m1[jh][:, (ih - jh) * 128:(ih - jh + 1) * 128],
                                start=False,
                                stop=(jh == ih),
                            )
                    spst = psAB.tile([128, 512], f32, tag="B5", name=f"sps{nm}")
                    sps_ps = spst[:, 0:128]
                    for jh in range(4):
                        nc.tensor.matmul(
                            sps_ps, kns[jh][:],
                            vN[:, c * 4 + jh, h * 128:(h + 1) * 128],
                            start=(jh == 0), stop=(jh == 3),
                        )
                    sq = bscr2.tile([128, B2], bf16, tag="SQ", bufs=2, name=f"sq{nm}")
                    nc.scalar.square(sq[:], ob[:])
                    xch = bscr2.tile([128, B2], bf16, tag=f"XC{h}", bufs=2, name=f"xc{nm}")
                    nc.vector.tensor_mul(xch[:], ob[:], gTb[:, h, c * B2:(c + 1) * B2])
                    nc.vector.scalar_tensor_tensor(
                        out=S[h][:],
                        in0=S[h][:],
                        scalar=bd_sb[:, h: h + 1],
                        in1=sps_ps,
                        op0=ALU.mult,
                        op1=ALU.add,
                    )
                    obq.append(sq)
                    return xch

                for t in range(NT):
                    emit_slice(t)
                    if t == 0:
                        # table loads ride the Act queue once the h/w stream
                        # for the first slice has been issued on SP
                        nc.scalar.dma_start(mk_sb[:], maskT[:, :])
                        nc.scalar.dma_start(qd_sb[:], qdtab[:, :])
                        nc.scalar.dma_start(kd_sb[:], kdcol[:, :])
                        nc.scalar.dma_start(bd_sb[:], bdcol[:, :])
                        nc.scalar.dma_start(wd_sb[:], wdd[:, :, :])

                prev_xc = None
                for c in range(NCH):
                    obq = []
                    pts0, tkl0 = scores_stage(c, 0)
                    m10 = mask_stage(c, 0, pts0)
                    kns0 = kns_stage(c, 0, tkl0)
                    qp0 = qp_stage(c, 0)
                    if prev_xc is not None:
                        dense_half(prev_xc[0], prev_xc[1], [0, 1])
                    pts1, tkl1 = scores_stage(c, 1)
                    m11 = mask_stage(c, 1, pts1)
                    kns1 = kns_stage(c, 1, tkl1)
                    qp1 = qp_stage(c, 1)
                    if prev_xc is not None:
                        dense_half(prev_xc[0], prev_xc[1], [2, 3])
                    xc0 = out_stage(c, 0, m10, kns0, qp0, obq)
                    xc1 = out_stage(c, 1, m11, kns1, qp1, obq)
                    # ssq: accumulate both heads via ones-matmul (B5 bank reuse)
                    sqps = psAB.tile([1, 512], f32, tag="B5", name=f"sqps{c}")
                    for h in range(HPR):
                        nc.tensor.matmul(
                            sqps[:], onesb[:], obq[h][:],
                            start=(h == 0), stop=(h == HPR - 1),
                        )
                    ssqt = bscr2.tile([1, 512], f32, tag="SSQ", bufs=1, name=f"ssqt{c}")
                    nc.scalar.copy(ssqt[:], sqps[:])
                    nc.sync.dma_start(ssq[:, c * B2:(c + 1) * B2], ssqt[:])
                    prev_xc = (c, [xc0, xc1])

                dense_half(prev_xc[0], prev_xc[1], [0, 1])
                dense_half(prev_xc[0], prev_xc[1], [2, 3])

    nc.compile()
    return nc


def _slopes(n):
    start = 2.0 ** (-(2.0 ** -(np.log2(n) - 3)))
    return np.array([start ** (i + 1) for i in range(n)], dtype=np.float64)


def kernel(hidden_states, positions, w_qkv, w_g, w_dense, g_norm_weight):
    global _PROGRAM
    if _PROGRAM is None:
        _PROGRAM = _build_program()
    nc = _PROGRAM

    hidden_states = np.asarray(hidden_states, dtype=np.float32)
    positions = np.asarray(positions)
    w_qkv = np.asarray(w_qkv, dtype=np.float32)
    w_g = np.asarray(w_g, dtype=np.float32)
    w_dense = np.asarray(w_dense, dtype=np.float32)
    g_norm_weight = np.asarray(g_norm_weight, dtype=np.float32)

    hT = hidden_states.T.reshape(NK, 128, T).transpose(1, 0, 2)
    hTb = np.ascontiguousarray(hT).astype(NPBF16)

    # rope tables, feature-major; sinT carries the rotate-half signs
    half = D // 2
    inv_freq = 1.0 / (THETA ** (np.arange(0, D, 2, dtype=np.float64) / D))
    freqs = positions.astype(np.float64)[:, None] * inv_freq          # [T, 64]
    cos = np.cos(freqs).T                                             # [64, T]
    sin = np.sin(freqs).T
    cosT = np.concatenate([cos, cos], axis=0).astype(NPBF16)          # [128, T]
    sinT = np.concatenate([sin, -sin], axis=0).astype(NPBF16)

    s = _slopes(H) * (1.0 - LAYER_ID / (NUM_LAYERS - 1) + 1e-5)       # [16]
    idx = np.arange(B2, dtype=np.float64)
    diff = idx[:, None] - idx[None, :]
    scale = D ** -0.5
    decay = np.where(
        diff[None, :, :] >= 0, np.exp(-s[:, None, None] * diff[None, :, :]), 0.0
    )                                                                  # [16, B2, B2]
    qd = np.exp(-s[:, None] * (idx[None, :] + 1.0)) * scale            # [16, B2]
    kd = np.exp(-s[:, None] * (B2 - 1.0 - idx[None, :]))               # [16, B2]
    bd = np.exp(-s * B2)                                               # [16]

    in_maps = []
    for r in range(M):
        heads = [HPR * r + i for i in range(HPR)]
        cols = slice(r * CW, (r + 1) * CW)
        wq = w_qkv[:, r * CW:(r + 1) * CW]
        wk = w_qkv[:, HID + r * CW: HID + (r + 1) * CW]
        wv = w_qkv[:, 2 * HID + r * CW: 2 * HID + (r + 1) * CW]
        wg = w_g[:, cols]
        wcat = np.concatenate([wq, wk, wg, wv], axis=1)               # [HID, 1024]
        w_allr = np.ascontiguousarray(
            wcat.reshape(NK, 128, 1024).transpose(1, 0, 2)
        ).astype(NPBF16)
        wdr = (g_norm_weight[cols, None] * w_dense[cols, :])
        wdr = np.ascontiguousarray(
            wdr.reshape(HPR, 128, HID).transpose(1, 0, 2)
        ).astype(NPBF16)

        mk = np.empty((128, HPR * 4 * B2), np.float64)
        qdt = np.empty((128, HPR * B2), np.float64)
        kdc = np.empty((128, HPR * 4), np.float32)
        bdc = np.empty((128, HPR), np.float32)
        for i, h in enumerate(heads):
            mTh = decay[h].T * scale                                   # [j, i]
            for jh in range(4):
                mk[:, (i * 4 + jh) * B2:(i * 4 + jh + 1) * B2] = (
                    mTh[jh * 128:(jh + 1) * 128, :]
                )
                kdc[:, i * 4 + jh] = kd[h, jh * 128:(jh + 1) * 128]
            qdt[:, i * B2:(i + 1) * B2] = np.broadcast_to(
                qd[h][None, :], (128, B2)
            )
            bdc[:, i] = bd[h]

        in_maps.append(
            {
                "hTb": hTb,
                "w_all": w_allr,
                "wdd": wdr,
                "cosT": cosT,
                "sinT": sinT,
                "maskT": mk.astype(NPBF16),
                "qdtab": qdt.astype(NPBF16),
                "kdcol": kdc,
                "bdcol": bdc,
            }
        )

    global _LAST_IN_MAPS
    _LAST_IN_MAPS = in_maps
    results = bass_utils.run_bass_kernel_spmd(nc, in_maps, core_ids=list(range(M)))

    y_sum = np.zeros((T, HID), np.float64)
    ssq_tot = np.zeros((T,), np.float64)
    for r in range(M):
        y_sum += results.results[r]["y_nat"].astype(np.float64)
        ssq_tot += results.results[r]["ssq"][0].astype(np.float64)
    var = ssq_tot / (H * D)
    F = 1.0 / np.sqrt(var + EPS)
    y = y_sum * F[:, None]
    return y.astype(np.float32)


# revision 21
# speedup vs baseline: 1.0261x; 1.0118x over previous
"""BailingMoE linear attention block on 8 trn2 cores (tensor-parallel over heads).

v3: bf16 datapath, single-pass projections, and slice/chunk interleaving.

Per rank r of 8 (heads 2r, 2r+1; 256 of 2048 feature columns):
  - one pass over hidden^T (bf16): feature-major q,k,g projections and
    natural-layout v projection from the same SBUF-resident h tiles.
  - RoPE: k on DVE straight out of PSUM, q on Pool from an Act-drained
    scratch copy (Pool cannot read PSUM).
  - chunked linear attention (B=512) interleaved with the projections:
    slice c is projected, then chunk c's attention runs while slice c+1's
    matmuls keep the PE busy, so the PSUM-drain engines (DVE/Act) are never
    the critical path. The 8 PSUM banks are shared between the two roles
    via common pool tags.
  - triangular-trimmed scores/outputs, PE transposes of k for the state
    update, bf16 small matmuls, per-head decay state S [128,128] bf16.
  - fused dense (row slice of w_dense, g_norm pre-folded), y partial bf16;
    dense for chunk c-1 is emitted inside chunk c (software pipeline).
  - partial sum-of-squares of o emitted as ssq [1, T] f32.
Host: y = (sum_r y_r) * rsqrt(sum_r ssq_r / 2048 + eps) -- the RMSNorm scale
commutes through the dense matmul, so no on-device collective is needed.
"""
import numpy as np
import ml_dtypes
import concourse.bass as bass
import concourse.mybir as mybir
import concourse.tile as tile
from concourse import bacc, bass_utils

T, HID, H, D = 4096, 2048, 16, 128
EPS = 1e-5
THETA = 600000.0
NUM_LAYERS, LAYER_ID = 32, 0
M = 8                 # cores
HPR = H // M          # heads per rank = 2
CW = HPR * D          # feature columns per rank = 256
NK = HID // 128       # 16 k-tiles
TT = 512              # t-slice for projections
NT = T // TT          # 8
B2 = 512              # attention chunk
NCH = T // B2         # 8

f32 = mybir.dt.float32
bf16 = mybir.dt.bfloat16
NPBF16 = ml_dtypes.bfloat16

_PROGRAM = None


def _build_program():
    nc = bacc.Bacc(trn_type="TRN2")

    hTb = nc.dram_tensor("hTb", [128, NK, T], bf16, kind="ExternalInput")
    w_all = nc.dram_tensor("w_all", [128, NK, 1024], bf16, kind="ExternalInput")
    wdd = nc.dram_tensor("wdd", [128, HPR, HID], bf16, kind="ExternalInput")
    cosT = nc.dram_tensor("cosT", [D, T], bf16, kind="ExternalInput")
    sinT = nc.dram_tensor("sinT", [D, T], bf16, kind="ExternalInput")
    maskT = nc.dram_tensor("maskT", [128, HPR * 4 * B2], bf16, kind="ExternalInput")
    qdtab = nc.dram_tensor("qdtab", [128, HPR * B2], bf16, kind="ExternalInput")
    kdcol = nc.dram_tensor("kdcol", [128, HPR * 4], f32, kind="ExternalInput")
    bdcol = nc.dram_tensor("bdcol", [128, HPR], f32, kind="ExternalInput")

    y_nat = nc.dram_tensor("y_nat", [T, HID], bf16, kind="ExternalOutput")
    ssq = nc.dram_tensor("ssq", [1, T], f32, kind="ExternalOutput")

    ACT = mybir.ActivationFunctionType
    ALU = mybir.AluOpType
    half = D // 2

    with tile.TileContext(nc) as tc:
        with tc.tile_pool(name="persist", bufs=1) as persist:
            qTb = persist.tile([128, HPR, T], bf16, name="qTb")
            kTb = persist.tile([128, HPR, T], bf16, name="kTb")
            vN = persist.tile([128, NCH * 4, CW], bf16, name="vN")
            gTb = persist.tile([128, HPR, T], bf16, name="gTb")
            S = [persist.tile([128, 128], bf16, name=f"S{h}") for h in range(HPR)]
            mk_sb = persist.tile([128, HPR * 4 * B2], bf16, name="mk_sb")
            qd_sb = persist.tile([128, HPR * B2], bf16, name="qd_sb")
            kd_sb = persist.tile([128, HPR * 4], f32, name="kd_sb")
            bd_sb = persist.tile([128, HPR], f32, name="bd_sb")
            wd_sb = persist.tile([128, HPR, HID], bf16, name="wd_sb")
            ident_f = persist.tile([128, 128], f32, name="ident_f")
            identb = persist.tile([128, 128], bf16, name="identb")
            onesb = persist.tile([128, 1], bf16, name="onesb")

            from concourse.masks import make_identity

            make_identity(nc, ident_f[:])
            nc.vector.tensor_copy(identb[:], ident_f[:])
            nc.gpsimd.memset(onesb[:], 1.0)
            for h in range(HPR):
                nc.vector.memset(S[h][:], 0.0)

            with (
                tc.tile_pool(name="work", bufs=1) as work,
                tc.tile_pool(name="psAB", bufs=1, space="PSUM") as psAB,
            ):
                wpool = astream = ascr = bscr2 = work
                w_sb = wpool.tile([128, NK, 1024], bf16, name="w_sb")

                # ---------------- phase A slice ------------------------------
                def emit_slice(t):
                    tsl = bass.ds(t * TT, TT)
                    hk = astream.tile([128, NK, TT], bf16, tag="hk", bufs=2, name=f"hk{t}")
                    if t == 0:
                        # first slice: per-k h tiles on SP, per-k w tiles on
                        # Act, so both queues stream in parallel and the
                        # k-major matmul order below starts after ~0.4 MB
                        for k in range(NK):
                            nc.sync.dma_start(
                                hk[:, k:k + 1, :],
                                hTb[:, k:k + 1, t * TT:(t + 1) * TT],
                            )
                            nc.scalar.dma_start(
                                w_sb[:, k:k + 1, :], w_all[:, k:k + 1, :]
                            )
                    else:
                        for k4 in range(0, NK, 4):
                            nc.sync.dma_start(
                                hk[:, k4:k4 + 4, :],
                                hTb[:, k4:k4 + 4, t * TT:(t + 1) * TT],
                            )
                    cs = astream.tile([128, TT], bf16, tag="cs", bufs=2, name=f"cs{t}")
                    nc.scalar.dma_start(cs[:], cosT[:, t * TT:(t + 1) * TT])
                    sn = astream.tile([128, TT], bf16, tag="sn", bufs=2, name=f"sn{t}")
                    nc.scalar.dma_start(sn[:], sinT[:, t * TT:(t + 1) * TT])

                    # feature-major: q0 q1 k0 k1 g0 g1 on bank tags B0..B5
                    accs = [
                        psAB.tile([128, TT], f32, tag=f"B{fg}", name=f"A{t}_{fg}")
                        for fg in range(6)
                    ]
                    vb0 = psAB.tile([128, 512], f32, tag="B6", name=f"VB0_{t}")
                    vb1 = psAB.tile([128, 512], f32, tag="B7", name=f"VB1_{t}")
                    accv = [
                        vb0[:, 0:256], vb0[:, 256:512],
                        vb1[:, 0:256], vb1[:, 256:512],
                    ]
                    if t == 0:
                        # k-major: each k-step only needs hk[k] + w[k].
                        # (v matmuls stay fg-major below: accv pairs share a
                        # PSUM bank, and accumulation groups in one bank
                        # cannot interleave.)
                        for k in range(NK):
                            for fg in range(6):
                                nc.tensor.matmul(
                                    accs[fg][:],
                                    w_sb[:, k, fg * 128:(fg + 1) * 128],
                                    hk[:, k, :],
                                    start=(k == 0),
                                    stop=(k == NK - 1),
                                )
                    for fg in range(6):
                        acc = accs[fg]
                        if t > 0:
                            for k in range(NK):
                                nc.tensor.matmul(
                                    acc[:],
                                    w_sb[:, k, fg * 128:(fg + 1) * 128],
                                    hk[:, k, :],
                                    start=(k == 0),
                                    stop=(k == NK - 1),
                                )
                        if fg < 4:
                            # rope via Act-drained scratch: q on DVE, k on Pool
                            h = fg % 2
                            scr = ascr.tile(
                                [128, TT], bf16, tag=f"scr{fg}", bufs=2,
                                name=f"scr{t}_{fg}"
                            )
                            nc.scalar.copy(scr[:], acc[:])
                            eng = nc.vector if fg < 2 else nc.gpsimd
                            en = "v" if fg < 2 else "p"
                            dst = qTb if fg < 2 else kTb
                            t1 = ascr.tile(
                                [128, TT], bf16, tag=f"t1{en}", name=f"t1{t}_{fg}"
                            )
                            tmp = ascr.tile(
                                [128, TT], bf16, tag=f"tm{en}", name=f"tm{t}_{fg}"
                            )
                            eng.tensor_mul(t1[:], scr[:], cs[:])
                            eng.tensor_mul(
                                tmp[0:half, :], scr[half:D, :], sn[half:D, :]
                            )
                            eng.tensor_mul(
                                tmp[half:D, :], scr[0:half, :], sn[0:half, :]
                            )
                            eng.tensor_add(dst[:, h, tsl], t1[:], tmp[:])
                        else:
                            nc.scalar.activation(
                                gTb[:, fg - 4, tsl], acc[:], ACT.Sigmoid
                            )

                    # natural-layout v (both heads' 256 features at once)
                    for tb in range(4):
                        for k in range(NK):
                            nc.tensor.matmul(
                                accv[tb],
                                hk[:, k, tb * 128:(tb + 1) * 128],
                                w_sb[:, k, 768:1024],
                                start=(k == 0),
                                stop=(k == NK - 1),
                            )
                        nc.scalar.copy(vN[:, t * 4 + tb, :], accv[tb])

                # ---------------- phase B chunk ------------------------------
                def scores_stage(c, h):
                    """PE: scores (triangular) + k transposes."""
                    nm = f"{c}_{h}"
                    qs = qTb[:, h, c * B2:(c + 1) * B2]
                    pta = psAB.tile([128, 512], f32, tag="B0", name=f"pta{nm}")
                    ptb = psAB.tile([128, 512], f32, tag="B1", name=f"ptb{nm}")
                    ptc = psAB.tile([128, 512], f32, tag="B2", name=f"ptc{nm}")
                    tkd = psAB.tile([128, 512], bf16, tag="B3", name=f"tkd{nm}")
                    pts = [
                        pta[:, 0:512], ptb[:, 0:384],
                        ptc[:, 0:256], ptb[:, 384:512],
                    ]
                    tkl = [tkd[:, jh * 128:(jh + 1) * 128] for jh in range(4)]
                    for jh in range(4):
                        nc.tensor.matmul(
                            pts[jh],
                            kTb[:, h, c * B2 + jh * 128: c * B2 + (jh + 1) * 128],
                            qs[:, jh * 128:B2],
                            start=True,
                            stop=True,
                        )
                    for jh in range(4):
                        nc.tensor.transpose(
                            tkl[jh],
                            kTb[:, h, c * B2 + jh * 128: c * B2 + (jh + 1) * 128],
                            identb[:],
                        )
                    return pts, tkl

                def mask_stage(c, h, pts):
                    """DVE: masked scores -> bf16 (trimmed)."""
                    nm = f"{c}_{h}"
                    m1 = []
                    for jh in range(4):
                        w = B2 - jh * 128
                        m1t = bscr2.tile(
                            [128, w], bf16, tag=f"M1_{jh}", bufs=2,
                            name=f"m1{nm}_{jh}"
                        )
                        nc.vector.tensor_mul(
                            m1t[:, 0:w],
                            pts[jh],
                            mk_sb[:, (h * 4 + jh) * B2 + jh * 128:
                                  (h * 4 + jh + 1) * B2],
                        )
                        m1.append(m1t)
                    return m1

                def kns_stage(c, h, tkl):
                    """Act: scaled natural-layout k -> bf16."""
                    nm = f"{c}_{h}"
                    kns = []
                    for jh in range(4):
                        knt = bscr2.tile(
                            [128, 128], bf16, tag=f"KN_{jh}", bufs=2,
                            name=f"kn{nm}_{jh}"
                        )
                        nc.scalar.activation(
                            knt[:], tkl[jh], ACT.Copy,
                            scale=kd_sb[:, h * 4 + jh: h * 4 + jh + 1],
                        )
                        kns.append(knt)
                    return kns

                def qp_stage(c, h):
                    nm = f"{c}_{h}"
                    qp = bscr2.tile([128, B2], bf16, tag=f"QP{h}", bufs=2, name=f"qp{nm}")
                    nc.gpsimd.tensor_mul(
                        qp[:], qTb[:, h, c * B2:(c + 1) * B2],
                        qd_sb[:, h * B2:(h + 1) * B2],
                    )
                    return qp

                def dense_half(cp, xc, tbs):
                    """Dense for chunk cp over token subtiles tbs."""
                    for tb in tbs:
                        dsb = bscr2.tile(
                            [128, HID], bf16, tag=f"DSB{tb}", bufs=1,
                            name=f"dsb{cp}_{tb}",
                        )
                        for ms in range(4):
                            dps = psAB.tile(
                                [128, 512], f32, tag=f"B{6 + ms % 2}",
                                name=f"d{cp}_{tb}_{ms}",
                            )
                            for kf in range(HPR):
                                nc.tensor.matmul(
                                    dps[:],
                                    xc[kf][:, tb * 128:(tb + 1) * 128],
                                    wd_sb[:, kf, ms * 512:(ms + 1) * 512],
                                    start=(kf == 0),
                                    stop=(kf == HPR - 1),
                                )
                            idx = tb * 4 + ms
                            if idx in (0, 3, 6, 9, 12, 15):
                                nc.vector.tensor_copy(
                                    dsb[:, ms * 512:(ms + 1) * 512], dps[:]
                                )
                            else:
                                nc.scalar.copy(
                                    dsb[:, ms * 512:(ms + 1) * 512], dps[:]
                                )
                            if cp == NCH - 1:
                                nc.sync.dma_start(
                                    y_nat[(cp * 4 + tb) * 128:
                                          (cp * 4 + tb + 1) * 128,
                                          ms * 512:(ms + 1) * 512],
                                    dsb[:, ms * 512:(ms + 1) * 512],
                                )
                        if cp != NCH - 1:
                            nc.sync.dma_start(
                                y_nat[(cp * 4 + tb) * 128:(cp * 4 + tb + 1) * 128, :],
                                dsb[:],
                            )

                def out_stage(c, h, m1, kns, qp, obq):
                    """PE o + sps; Act sq; DVE xch + S update."""
                    nm = f"{c}_{h}"
                    ob = psAB.tile([128, 512], f32, tag="B4", name=f"ob{nm}")
                    for ih in range(4):
                        osl = ob[:, ih * 128:(ih + 1) * 128]
                        nc.tensor.matmul(
                            osl, S[h][:], qp[:, ih * 128:(ih + 1) * 128],
                            start=True, stop=False,
                        )
                        for jh in range(ih + 1):
                            nc.tensor.matmul(
                                osl,
                                vN[:, c * 4 + jh, h * 128:(h + 1) * 128],
                                m1[jh][:, (ih - jh) * 128:(ih - jh + 1) * 128],
                                start=False,
                                stop=(jh == ih),
                            )
                    spst = psAB.tile([128, 512], f32, tag="B5", name=f"sps{nm}")
                    sps_ps = spst[:, 0:128]
                    for jh in range(4):
                        nc.tensor.matmul(
                            sps_ps, kns[jh][:],
                            vN[:, c * 4 + jh, h * 128:(h + 1) * 128],
                            start=(jh == 0), stop=(jh == 3),
                        )
                    sq = bscr2.tile([128, B2], bf16, tag="SQ", bufs=2, name=f"sq{nm}")
                    nc.scalar.square(sq[:], ob[:])
                    xch = bscr2.tile([128, B2], bf16, tag=f"XC{h}", bufs=2, name=f"xc{nm}")
                    nc.vector.tensor_mul(xch[:], ob[:], gTb[:, h, c * B2:(c + 1) * B2])
                    nc.vector.scalar_tensor_tensor(
                        out=S[h][:],
                        in0=S[h][:],
                        scalar=bd_sb[:, h: h + 1],
                        in1=sps_ps,
                        op0=ALU.mult,
                        op1=ALU.add,
                    )
                    obq.append(sq)
                    return xch

                for t in range(NT):
                    emit_slice(t)
                    if t == 0:
                        # table loads ride the Act queue once the h/w stream
                        # for the first slice has been issued on SP
                        nc.scalar.dma_start(mk_sb[:], maskT[:, :])
                        nc.scalar.dma_start(qd_sb[:], qdtab[:, :])
                        nc.scalar.dma_start(kd_sb[:], kdcol[:, :])
                        nc.scalar.dma_start(bd_sb[:], bdcol[:, :])
                        nc.scalar.dma_start(wd_sb[:], wdd[:, :, :])

                prev_xc = None
                for c in range(NCH):
                    obq = []
                    last = c == NCH - 1
                    pts0, tkl0 = scores_stage(c, 0)
                    m10 = mask_stage(c, 0, pts0)
                    kns0 = kns_stage(c, 0, tkl0)
                    qp0 = qp_stage(c, 0)
                    if prev_xc is not None:
                        dense_half(prev_xc[0], prev_xc[1], [0, 1])
                    if last:
                        xc0 = out_stage(c, 0, m10, kns0, qp0, obq)
                    pts1, tkl1 = scores_stage(c, 1)
                    m11 = mask_stage(c, 1, pts1)
                    kns1 = kns_stage(c, 1, tkl1)
                    qp1 = qp_stage(c, 1)
                    if prev_xc is not None:
                        dense_half(prev_xc[0], prev_xc[1], [2, 3])
                    if not last:
                        xc0 = out_stage(c, 0, m10, kns0, qp0, obq)
                    xc1 = out_stage(c, 1, m11, kns1, qp1, obq)
                    # ssq: accumulate both heads via ones-matmul (B5 bank reuse)
                    sqps = psAB.tile([1, 512], f32, tag="B5", name=f"sqps{c}")
                    for h in range(HPR):
                        nc.tensor.matmul(
                            sqps[:], onesb[:], obq[h][:],
                            start=(h == 0), stop=(h == HPR - 1),
                        )
                    ssqt = bscr2.tile([1, 512], f32, tag="SSQ", bufs=1, name=f"ssqt{c}")
                    nc.scalar.copy(ssqt[:], sqps[:])
                    nc.sync.dma_start(ssq[:, c * B2:(c + 1) * B2], ssqt[:])
                    prev_xc = (c, [xc0, xc1])

                dense_half(prev_xc[0], prev_xc[1], [0, 1])
                dense_half(prev_xc[0], prev_xc[1], [2, 3])

    nc.compile()
    return nc


def _slopes(n):
    start = 2.0 ** (-(2.0 ** -(np.log2(n) - 3)))
    return np.array([start ** (i + 1) for i in range(n)], dtype=np.float64)


def kernel(hidden_states, positions, w_qkv, w_g, w_dense, g_norm_weight):
    global _PROGRAM
    if _PROGRAM is None:
        _PROGRAM = _build_program()
    nc = _PROGRAM

    hidden_states = np.asarray(hidden_states, dtype=np.float32)
    positions = np.asarray(positions)
    w_qkv = np.asarray(w_qkv, dtype=np.float32)
    w_g = np.asarray(w_g, dtype=np.float32)
    w_dense = np.asarray(w_dense, dtype=np.float32)
    g_norm_weight = np.asarray(g_norm_weight, dtype=np.float32)

    hT = hidden_states.T.reshape(NK, 128, T).transpose(1, 0, 2)
    hTb = np.ascontiguousarray(hT).astype(NPBF16)

    # rope tables, feature-major; sinT carries the rotate-half signs
    half = D // 2
    inv_freq = 1.0 / (THETA ** (np.arange(0, D, 2, dtype=np.float64) / D))
    freqs = positions.astype(np.float64)[:, None] * inv_freq          # [T, 64]
    cos = np.cos(freqs).T                                             # [64, T]
    sin = np.sin(freqs).T
    cosT = np.concatenate([cos, cos], axis=0).astype(NPBF16)          # [128, T]
    sinT = np.concatenate([sin, -sin], axis=0).astype(NPBF16)

    s = _slopes(H) * (1.0 - LAYER_ID / (NUM_LAYERS - 1) + 1e-5)       # [16]
    idx = np.arange(B2, dtype=np.float64)
    diff = idx[:, None] - idx[None, :]
    scale = D ** -0.5
    decay = np.where(
        diff[None, :, :] >= 0, np.exp(-s[:, None, None] * diff[None, :, :]), 0.0
    )                                                                  # [16, B2, B2]
    qd = np.exp(-s[:, None] * (idx[None, :] + 1.0)) * scale            # [16, B2]
    kd = np.exp(-s[:, None] * (B2 - 1.0 - idx[None, :]))               # [16, B2]
    bd = np.exp(-s * B2)                                               # [16]

    in_maps = []
    for r in range(M):
        heads = [HPR * r + i for i in range(HPR)]
        cols = slice(r * CW, (r + 1) * CW)
        wq = w_qkv[:, r * CW:(r + 1) * CW]
        wk = w_qkv[:, HID + r * CW: HID + (r + 1) * CW]
        wv = w_qkv[:, 2 * HID + r * CW: 2 * HID + (r + 1) * CW]
        wg = w_g[:, cols]
        wcat = np.concatenate([wq, wk, wg, wv], axis=1)               # [HID, 1024]
        w_allr = np.ascontiguousarray(
            wcat.reshape(NK, 128, 1024).transpose(1, 0, 2)
        ).astype(NPBF16)
        wdr = (g_norm_weight[cols, None] * w_dense[cols, :])
        wdr = np.ascontiguousarray(
            wdr.reshape(HPR, 128, HID).transpose(1, 0, 2)
        ).astype(NPBF16)

        mk = np.empty((128, HPR * 4 * B2), np.float64)
        qdt = np.empty((128, HPR * B2), np.float64)
        kdc = np.empty((128, HPR * 4), np.float32)
        bdc = np.empty((128, HPR), np.float32)
        for i, h in enumerate(heads):
            mTh = decay[h].T * scale                                   # [j, i]
            for jh in range(4):
                mk[:, (i * 4 + jh) * B2:(i * 4 + jh + 1) * B2] = (
                    mTh[jh * 128:(jh + 1) * 128, :]
                )
                kdc[:, i * 4 + jh] = kd[h, jh * 128:(jh + 1) * 128]
            qdt[:, i * B2:(i + 1) * B2] = np.broadcast_to(
                qd[h][None, :], (128, B2)
            )
            bdc[:, i] = bd[h]

        in_maps.append(
            {
                "hTb": hTb,
                "w_all": w_allr,
                "wdd": wdr,
                "cosT": cosT,
                "sinT": sinT,
                "maskT": mk.astype(NPBF16),
                "qdtab": qdt.astype(NPBF16),
                "kdcol": kdc,
                "bdcol": bdc,
            }
        )

    global _LAST_IN_MAPS
    _LAST_IN_MAPS = in_maps
    results = bass_utils.run_bass_kernel_spmd(nc, in_maps, core_ids=list(range(M)))

    y_sum = np.zeros((T, HID), np.float64)
    ssq_tot = np.zeros((T,), np.float64)
    for r in range(M):
        y_sum += results.results[r]["y_nat"].astype(np.float64)
        ssq_tot += results.results[r]["ssq"][0].astype(np.float64)
    var = ssq_tot / (H * D)
    F = 1.0 / np.sqrt(var + EPS)
    y = y_sum * F[:, None]
    return y.astype(np.float32)
